# revision 10
# baseline (speedup 1.0000x reference)
"""AECNF (autoencoder + conditional CNF) forward pass on 8 TRN2 NeuronCores.

Strategy: pure data parallelism — batch 8192 is split into 8 shards of 1024,
one per core; all parameters are replicated. On-device everything is kept
feature-major ([features, batch]) so the whole network is a chain of
matmuls with no transposes: out = W @ X via nc.tensor.matmul(psum, lhsT, rhs)
with lhsT = W.T (features on partitions).

The HyperNetwork depends only on t (33 unique compile-time-constant values),
not on the batch, so its outputs (Wm, Um*sigmoid(gate), Bm, wu) are
precomputed on the host as weight preprocessing and fed to the device as
packed constant tensors. The condition path dc/dt = c is linear, so
c(t) = c_emb * factor(t) with host-precomputed f32 scalar factors folded
into the per-eval matmul weights.

The log-det trace term accumulates across all 64 ODE evals directly in a
persistent PSUM bank: logp_x = PSUM(sum_e -cw_e/256 * wu_e . hh_e^2
                                     - 0.5 * 1 . z0^2) + CONST.
"""
import sys
import numpy as np

if '/opt/trn_rl_repo' not in sys.path:
    sys.path.insert(0, '/opt/trn_rl_repo')

# ---- problem dims (hardcoded per spec) ----
B = 8192
NCORES = 8
BL = B // NCORES          # 1024 per core
N = 512                   # batch chunk (PSUM free-dim limit)
NCH = BL // N             # 2 chunks
D_IN = 784
H = 512
LD = 16
CD = 32
WID = 256
OH = 256
NSTEPS = 16
KC0 = 7                   # ceil(784/128)
D_PAD = KC0 * 128         # 896
MC0 = H // 128            # 4
KC1 = H // 128            # 4
MC2 = KC0                 # 7 output chunks for decoder L3
NU = 2 * NSTEPS + 1       # 33 unique hypernet t values
NE = 4 * NSTEPS           # 64 ODE function evals

_F32 = np.float32

_CACHE = {}
TRACE = False          # set by test harness to capture a neuron-profile trace
LAST_EXEC_NS = None
LAST_RES = None


def _f32(x):
    return np.asarray(x, dtype=np.float32)


def _sigmoid32(x):
    return (1.0 / (1.0 + np.exp(-x.astype(np.float32)))).astype(np.float32)


def _hypernet(t, hw1, hb1, hw2, hb2, hw3, hb3):
    """Host replica of the reference HyperNetwork at scalar t (float32)."""
    x = np.tanh(t * hw1[:, 0] + hb1).astype(np.float32)
    x = np.tanh(x @ hw2.T + hb2).astype(np.float32)
    p = (x @ hw3.T + hb3).astype(np.float32)
    bs = WID * LD
    Wm = p[:bs].reshape(WID, LD)
    Um = (p[bs:2 * bs].reshape(WID, LD) * _sigmoid32(p[2 * bs:3 * bs].reshape(WID, LD))).astype(np.float32)
    Bm = p[3 * bs:]
    wu = np.sum(Wm * Um, axis=1, dtype=np.float32)
    return Wm, Um, Bm, wu


def _pad_ms_cols(wT):
    """[K, 32] -> [K, 48]: cols 0:16 kept, 16:32 zero, 32:48 = old 16:32."""
    out = np.zeros((wT.shape[0], 48), dtype=np.float32)
    out[:, 0:LD] = wT[:, 0:LD]
    out[:, 32:48] = wT[:, LD:2 * LD]
    return out


def _pad_ms_rows(b):
    out = np.zeros((48, 1), dtype=np.float32)
    out[0:LD, 0] = b[:LD]
    out[32:48, 0] = b[LD:2 * LD]
    return out


def _pack_k(wT, kc, m):
    """Pack a [K, M] lhsT (K possibly >128, zero-padded to kc*128) into
    [128, kc*m] so slice [:, c*m + j] gives K-chunk c."""
    K, M = wT.shape
    assert M == m
    buf = np.zeros((kc * 128, M), dtype=np.float32)
    buf[:K] = wT
    return np.ascontiguousarray(buf.reshape(kc, 128, M).transpose(1, 0, 2).reshape(128, kc * M))


def _build_consts(ins):
    """All host-side weight preprocessing. Returns dict of device arrays."""
    g = {k: _f32(v) for k, v in ins.items() if k != 'condition'}

    dt = (_F32(0.0) - _F32(10.0)) / _F32(NSTEPS)       # -0.625
    hdt = dt / _F32(2.0)
    sdt6 = dt / _F32(6.0)

    # unique hypernet ts: t_u = 10 + u * dt/2
    t_us = [_F32(10.0) + hdt * _F32(u) for u in range(NU)]
    hyper = [_hypernet(t, g['hw1'], g['hb1'], g['hw2'], g['hb2'], g['hw3'], g['hb3'])
             for t in t_us]

    def u_of(step, stage):
        return 2 * step + (0, 1, 1, 2)[stage]

    # c(t) factors: f32 sequential RK4 on dc/dt = c
    cfac = np.zeros(NE, dtype=np.float32)
    fn = _F32(1.0)
    for step in range(NSTEPS):
        f1 = fn
        f2 = fn + hdt * f1
        f3 = fn + hdt * f2
        f4 = fn + dt * f3
        cfac[step * 4 + 0] = f1
        cfac[step * 4 + 1] = f2
        cfac[step * 4 + 2] = f3
        cfac[step * 4 + 3] = f4
        fn = fn + sdt6 * (f1 + _F32(2.0) * f2 + _F32(2.0) * f3 + f4)

    # per-eval a-matmul weights: lhsT = [cwz | cwc*cfac].T, output cols padded
    # so the sigmoid half lands at PSUM partition 32 (PSUM reads must be
    # 32-partition aligned): cols 0:16 tanh-half, 16:32 zero, 32:48 sig-half.
    aW = np.zeros((CD + LD, NE * 48), dtype=np.float32)
    for e in range(NE):
        blk = np.zeros((CD + LD, 48), dtype=np.float32)
        blk[:LD, 0:LD] = g['cwz'].T[:, 0:LD]
        blk[LD:, 0:LD] = (g['cwc'] * cfac[e]).T[:, 0:LD]
        blk[:LD, 32:48] = g['cwz'].T[:, LD:]
        blk[LD:, 32:48] = (g['cwc'] * cfac[e]).T[:, LD:]
        aW[:, e * 48:(e + 1) * 48] = blk
    b0c = (g['cbz'] + g['cbc']).astype(np.float32)
    b0 = np.zeros((48, 1), dtype=np.float32)
    b0[0:LD, 0] = b0c[:LD]
    b0[32:48, 0] = b0c[LD:]

    # hypernet-derived packs
    hhW = np.zeros((LD, NU * WID), dtype=np.float32)
    hhB = np.zeros((128, NU * 2), dtype=np.float32)
    kW = np.zeros((128, NU * 2 * LD), dtype=np.float32)
    for u, (Wm, Um, Bm, _) in enumerate(hyper):
        hhW[:, u * WID:(u + 1) * WID] = Wm.T
        for h in range(2):
            hhB[:, u * 2 + h] = Bm[h * 128:(h + 1) * 128]
            kW[:, u * 32 + h * LD:(u + 1 - 1) * 32 + (h + 1) * LD] = \
                Um[h * 128:(h + 1) * 128] / _F32(WID)

    # logp accumulation weights + constant
    w_stage = [sdt6 * _F32(m) for m in (1.0, 2.0, 2.0, 1.0)]
    lpW = np.zeros((128, NE * 2), dtype=np.float32)
    CONST = _F32(0.0)
    for step in range(NSTEPS):
        for stage in range(4):
            e = step * 4 + stage
            u = u_of(step, stage)
            wu = hyper[u][3]
            sc = -(w_stage[stage] / _F32(WID))
            for h in range(2):
                lpW[:, e * 2 + h] = wu[h * 128:(h + 1) * 128] * sc
            S0 = np.sum(wu, dtype=np.float32)
            CONST = CONST + w_stage[stage] * S0 / _F32(WID)
    CONST = _F32(CONST - _F32(0.5 * LD) * _F32(np.log(2 * np.pi)))
    z0W = np.full((LD, 1), -0.5, dtype=np.float32)

    consts = {
        'ew0p': _pack_k(g['ew0'].T, KC0, H),
        'ew1p': _pack_k(g['ew1'].T, KC1, H),
        'ew2p': _pack_k(_pad_ms_cols(g['ew2'].T), KC1, 48),
        'eb0p': np.ascontiguousarray(g['eb0'].reshape(MC0, 128).T),
        'eb1p': np.ascontiguousarray(g['eb1'].reshape(MC0, 128).T),
        'eb2p': _pad_ms_rows(g['eb2']),
        'dw0p': np.ascontiguousarray(g['dw0'].T),           # [16, 512]
        'dw1p': _pack_k(g['dw1'].T, KC1, H),
        'dw2p': _pack_k(g['dw2'].T, KC1, D_IN),
        'db0p': np.ascontiguousarray(g['db0'].reshape(MC0, 128).T),
        'db1p': np.ascontiguousarray(g['db1'].reshape(MC0, 128).T),
        'db2p': np.ascontiguousarray(
            np.pad(g['db2'], (0, D_PAD - D_IN)).reshape(MC2, 128).T),
        'aW': aW, 'b0': b0, 'hhW': hhW, 'hhB': hhB, 'kW': kW,
        'lpW': lpW, 'z0W': z0W,
    }
    meta = {
        'CONST': float(CONST),
        'w_stage': [float(w) for w in w_stage],
        's_fac': [float(hdt), float(hdt), float(dt)],
        'u_of': u_of,
    }
    return consts, meta


def _build_nc(const_shapes, meta):
    import concourse.bass as bass
    import concourse.tile as tile
    from concourse import bacc, mybir

    AF = mybir.ActivationFunctionType
    ALU = mybir.AluOpType
    f32 = mybir.dt.float32

    nc = bacc.Bacc("TRN2", target_bir_lowering=False, debug=False,
                   num_devices=NCORES)

    ap = {}
    ap['xT'] = nc.dram_tensor("xT", [128, KC0 * BL], f32, kind="ExternalInput").ap()
    ap['epsT'] = nc.dram_tensor("epsT", [LD, BL], f32, kind="ExternalInput").ap()
    ap['cT'] = nc.dram_tensor("cT", [CD, BL], f32, kind="ExternalInput").ap()
    for k, shp in const_shapes.items():
        ap[k] = nc.dram_tensor(k, list(shp), f32, kind="ExternalInput").ap()
    ap['recT'] = nc.dram_tensor("recT", [D_IN, BL], f32, kind="ExternalOutput").ap()
    ap['meanT'] = nc.dram_tensor("meanT", [LD, BL], f32, kind="ExternalOutput").ap()
    ap['stdT'] = nc.dram_tensor("stdT", [LD, BL], f32, kind="ExternalOutput").ap()
    ap['lp'] = nc.dram_tensor("lp", [1, BL], f32, kind="ExternalOutput").ap()

    MM = nc.tensor.matmul
    ACT = nc.scalar.activation
    CONST = meta['CONST']
    w_stage = meta['w_stage']
    s_fac = meta['s_fac']
    u_of = meta['u_of']

    with tile.TileContext(nc) as tc:
        from contextlib import ExitStack
        with ExitStack() as ctx:
            wp = ctx.enter_context(tc.tile_pool(name="wts", bufs=1))
            sp = ctx.enter_context(tc.tile_pool(name="state", bufs=1))
            lpp = ctx.enter_context(tc.tile_pool(name="lp_ps", bufs=1, space="PSUM"))

            # ---- load all weights/constants once ----
            W = {}
            for k, shp in const_shapes.items():
                W[k] = wp.tile(list(shp), f32, tag=k, name=f"w_{k}")
                nc.sync.dma_start(out=W[k][:], in_=ap[k][:])

            # ---- persistent state ----
            X = sp.tile([CD + LD, BL], f32, tag="X", name="X")
            X2 = sp.tile([CD + LD, BL], f32, tag="X2", name="X2")
            zacc = [sp.tile([LD, N], f32, tag=f"zacc{n}", name=f"zacc{n}") for n in range(NCH)]
            z_sb = sp.tile([LD, BL], f32, tag="z_sb", name="z_sb")
            mean_sb = sp.tile([LD, BL], f32, tag="mean_sb", name="mean_sb")
            std_sb = sp.tile([LD, BL], f32, tag="std_sb", name="std_sb")
            eps_sb = sp.tile([LD, BL], f32, tag="eps_sb", name="eps_sb")
            lp_ps = [lpp.tile([1, N], f32, tag=f"lp{n}", name=f"lp{n}") for n in range(NCH)]

            nc.sync.dma_start(out=eps_sb[:], in_=ap['epsT'][:])
            nc.sync.dma_start(out=X[LD:, :], in_=ap['cT'][:])
            nc.sync.dma_start(out=X2[LD:, :], in_=ap['cT'][:])

            # ================= encoder =================
            with tc.tile_pool(name="encps", bufs=1, space="PSUM") as psE, \
                 tc.tile_pool(name="enc_sb", bufs=2) as esb, \
                 tc.tile_pool(name="acts", bufs=1) as actp:
                h0 = actp.tile([128, MC0 * BL], f32, tag="h0", name="h0")
                h1 = actp.tile([128, MC0 * BL], f32, tag="h1", name="h1")

                def elu_store(ps, bias_ap, dst):
                    ee = esb.tile([128, N], f32, tag="ee", name="ee", bufs=2)
                    ACT(ee[:], ps[:], AF.Exp, bias=bias_ap)
                    rr = esb.tile([128, N], f32, tag="rr", name="rr", bufs=2)
                    ACT(rr[:], ps[:], AF.Relu, bias=bias_ap)
                    t2 = esb.tile([128, N], f32, tag="t2", name="t2", bufs=2)
                    ACT(t2[:], ee[:], AF.Relu, scale=-1.0, bias=1.0)
                    nc.vector.tensor_sub(dst, rr[:], t2[:])

                # L1: h0 = elu(ew0 @ x + eb0)
                for n in range(NCH):
                    ps_l = [psE.tile([128, N], f32, tag=f"eps{m}", name=f"eps{m}_{n}")
                            for m in range(MC0)]
                    for kc in range(KC0):
                        xk = esb.tile([128, N], f32, tag="xk", name="xk", bufs=3)
                        nc.sync.dma_start(
                            out=xk[:], in_=ap['xT'][:, kc * BL + n * N: kc * BL + (n + 1) * N])
                        for m in range(MC0):
                            MM(ps_l[m][:], W['ew0p'][:, kc * H + m * 128: kc * H + (m + 1) * 128],
                               xk[:], start=(kc == 0), stop=(kc == KC0 - 1))
                    for m in range(MC0):
                        elu_store(ps_l[m], W['eb0p'][:, m:m + 1],
                                  h0[:, m * BL + n * N: m * BL + (n + 1) * N])

                # L2: h1 = tanh(ew1 @ h0 + eb1)
                for n in range(NCH):
                    ps_l = [psE.tile([128, N], f32, tag=f"eps{m}", name=f"eps2{m}_{n}")
                            for m in range(MC0)]
                    for kc in range(KC1):
                        for m in range(MC0):
                            MM(ps_l[m][:], W['ew1p'][:, kc * H + m * 128: kc * H + (m + 1) * 128],
                               h0[:, kc * BL + n * N: kc * BL + (n + 1) * N],
                               start=(kc == 0), stop=(kc == KC1 - 1))
                    for m in range(MC0):
                        ACT(h1[:, m * BL + n * N: m * BL + (n + 1) * N],
                            ps_l[m][:], AF.Tanh, bias=W['eb1p'][:, m:m + 1])

                # L3: ms = ew2 @ h1 + eb2; mean/std; z = mean + eps*std
                for n in range(NCH):
                    ps3 = psE.tile([48, N], f32, tag="eps_ms", name=f"eps_ms{n}")
                    for kc in range(KC1):
                        MM(ps3[:], W['ew2p'][:, kc * 48:(kc + 1) * 48],
                           h1[:, kc * BL + n * N: kc * BL + (n + 1) * N],
                           start=(kc == 0), stop=(kc == KC1 - 1))
                    ACT(mean_sb[:, n * N:(n + 1) * N], ps3[0:LD, :], AF.Identity,
                        bias=W['eb2p'][0:LD, 0:1])
                    ACT(std_sb[:, n * N:(n + 1) * N], ps3[32:48, :], AF.Exp,
                        bias=W['eb2p'][32:48, 0:1])
                nc.vector.tensor_scalar_add(std_sb[:], std_sb[:], 1e-6)
                nc.vector.tensor_mul(z_sb[:], eps_sb[:], std_sb[:])
                nc.vector.tensor_add(z_sb[:], z_sb[:], mean_sb[:])
                nc.vector.tensor_copy(X[0:LD, :], z_sb[:])
                nc.sync.dma_start(out=ap['meanT'][:], in_=mean_sb[:])
                nc.sync.dma_start(out=ap['stdT'][:], in_=std_sb[:])

            # ================= decoder =================
            with tc.tile_pool(name="decps", bufs=1, space="PSUM") as psD, \
                 tc.tile_pool(name="dec_sb", bufs=2) as dsb, \
                 tc.tile_pool(name="dacts", bufs=1) as dactp:
                d1 = dactp.tile([128, MC0 * BL], f32, tag="d1", name="d1")
                d2 = dactp.tile([128, MC0 * BL], f32, tag="d2", name="d2")

                def delu_store(ps, bias_ap, dst):
                    ee = dsb.tile([128, N], f32, tag="dee", name="dee", bufs=2)
                    ACT(ee[:], ps[:], AF.Exp, bias=bias_ap)
                    rr = dsb.tile([128, N], f32, tag="drr", name="drr", bufs=2)
                    ACT(rr[:], ps[:], AF.Relu, bias=bias_ap)
                    t2 = dsb.tile([128, N], f32, tag="dt2", name="dt2", bufs=2)
                    ACT(t2[:], ee[:], AF.Relu, scale=-1.0, bias=1.0)
                    nc.vector.tensor_sub(dst, rr[:], t2[:])

                # L1: d1 = elu(dw0 @ z + db0)   (K = 16, single chunk)
                for n in range(NCH):
                    for m in range(MC0):
                        ps = psD.tile([128, N], f32, tag=f"dps{m}", name=f"dps{m}_{n}")
                        MM(ps[:], W['dw0p'][:, m * 128:(m + 1) * 128],
                           z_sb[:, n * N:(n + 1) * N], start=True, stop=True)
                        delu_store(ps, W['db0p'][:, m:m + 1],
                                   d1[:, m * BL + n * N: m * BL + (n + 1) * N])

                # L2: d2 = tanh(dw1 @ d1 + db1)
                for n in range(NCH):
                    ps_l = [psD.tile([128, N], f32, tag=f"dps{m}", name=f"dps2{m}_{n}")
                            for m in range(MC0)]
                    for kc in range(KC1):
                        for m in range(MC0):
                            MM(ps_l[m][:], W['dw1p'][:, kc * H + m * 128: kc * H + (m + 1) * 128],
                               d1[:, kc * BL + n * N: kc * BL + (n + 1) * N],
                               start=(kc == 0), stop=(kc == KC1 - 1))
                    for m in range(MC0):
                        ACT(d2[:, m * BL + n * N: m * BL + (n + 1) * N],
                            ps_l[m][:], AF.Tanh, bias=W['db1p'][:, m:m + 1])

                # L3: rec = sigmoid(dw2 @ d2 + db2)
                for n in range(NCH):
                    for m in range(MC2):
                        mm = 128 if m < MC2 - 1 else D_IN - 128 * (MC2 - 1)
                        ps = psD.tile([128, N], f32, tag=f"dps{m % MC0}",
                                      name=f"dps3{m}_{n}")
                        for kc in range(KC1):
                            MM(ps[0:mm, :],
                               W['dw2p'][:, kc * D_IN + m * 128: kc * D_IN + m * 128 + mm],
                               d2[:, kc * BL + n * N: kc * BL + (n + 1) * N],
                               start=(kc == 0), stop=(kc == KC1 - 1))
                        rec = dsb.tile([128, N], f32, tag="rec", name=f"rec{m}_{n}", bufs=3)
                        ACT(rec[0:mm, :], ps[0:mm, :], AF.Sigmoid,
                            bias=W['db2p'][0:mm, m:m + 1])
                        nc.sync.dma_start(
                            out=ap['recT'][m * 128:m * 128 + mm, n * N:(n + 1) * N],
                            in_=rec[0:mm, :])

            # ================= ODE (RK4, 16 steps, 64 evals) =================
            with tc.tile_pool(name="psA", bufs=2, space="PSUM") as psA, \
                 tc.tile_pool(name="psH", bufs=2, space="PSUM") as psH, \
                 tc.tile_pool(name="psK", bufs=2, space="PSUM") as psK, \
                 tc.tile_pool(name="ode_sb", bufs=3) as tp, \
                 tc.tile_pool(name="hh_sb", bufs=4) as hp:

                for step in range(NSTEPS):
                    for stage in range(4):
                        e = step * 4 + stage
                        u = u_of(step, stage)
                        for n in range(NCH):
                            rhs_t = X if stage == 0 else X2
                            rhs = rhs_t[:, n * N:(n + 1) * N]
                            aps = psA.tile([48, N], f32, tag="aps", name=f"aps{e}_{n}")
                            MM(aps[:], W['aW'][:, e * 48:(e + 1) * 48], rhs,
                               start=True, stop=True)
                            t1 = tp.tile([LD, N], f32, tag="t1", name=f"t1_{e}_{n}")
                            ACT(t1[:], aps[0:LD, :], AF.Tanh, bias=W['b0'][0:LD, 0:1])
                            s1 = tp.tile([LD, N], f32, tag="s1", name=f"s1_{e}_{n}")
                            ACT(s1[:], aps[32:48, :], AF.Sigmoid,
                                bias=W['b0'][32:48, 0:1])
                            zc = tp.tile([LD, N], f32, tag="zc", name=f"zc_{e}_{n}")
                            nc.vector.tensor_mul(zc[:], t1[:], s1[:])

                            kps = psK.tile([LD, N], f32, tag="kps", name=f"kps{e}_{n}")
                            for h in range(2):
                                hps = psH.tile([128, N], f32, tag="hps",
                                               name=f"hps{e}_{n}_{h}")
                                MM(hps[:], W['hhW'][:, u * WID + h * 128: u * WID + (h + 1) * 128],
                                   zc[:], start=True, stop=True)
                                hh = hp.tile([128, N], f32, tag="hh",
                                             name=f"hh{e}_{n}_{h}", bufs=4)
                                ACT(hh[:], hps[:], AF.Tanh,
                                    bias=W['hhB'][:, u * 2 + h: u * 2 + h + 1])
                                ss = hp.tile([128, N], f32, tag="ss",
                                             name=f"ss{e}_{n}_{h}", bufs=4)
                                nc.vector.tensor_mul(ss[:], hh[:], hh[:])
                                MM(kps[:], W['kW'][:, u * 32 + h * LD: u * 32 + (h + 1) * LD],
                                   hh[:], start=(h == 0), stop=(h == 1))
                                MM(lp_ps[n][:], W['lpW'][:, e * 2 + h: e * 2 + h + 1],
                                   ss[:], start=(e == 0 and h == 0), stop=False,
                                   skip_group_check=True)

                            Xz = X[0:LD, n * N:(n + 1) * N]
                            if stage == 0:
                                nc.vector.scalar_tensor_tensor(
                                    zacc[n][:], kps[:], w_stage[0], Xz,
                                    op0=ALU.mult, op1=ALU.add)
                            elif stage < 3:
                                nc.vector.scalar_tensor_tensor(
                                    zacc[n][:], kps[:], w_stage[stage], zacc[n][:],
                                    op0=ALU.mult, op1=ALU.add)
                            else:
                                nc.vector.scalar_tensor_tensor(
                                    Xz, kps[:], w_stage[3], zacc[n][:],
                                    op0=ALU.mult, op1=ALU.add)
                            if stage < 3:
                                nc.vector.scalar_tensor_tensor(
                                    X2[0:LD, n * N:(n + 1) * N], kps[:], s_fac[stage], Xz,
                                    op0=ALU.mult, op1=ALU.add)

                # ---- finish logp: -0.5*sum(z0^2), add CONST, write out ----
                for n in range(NCH):
                    sq = tp.tile([LD, N], f32, tag="sq", name=f"sq{n}", bufs=2)
                    nc.vector.tensor_mul(sq[:], X[0:LD, n * N:(n + 1) * N],
                                         X[0:LD, n * N:(n + 1) * N])
                    MM(lp_ps[n][:], W['z0W'][:], sq[:], start=False, stop=True,
                       skip_group_check=True)
                    lpo = tp.tile([1, N], f32, tag="lpo", name=f"lpo{n}", bufs=2)
                    ACT(lpo[:], lp_ps[n][:], AF.Copy, bias=CONST)
                    nc.sync.dma_start(out=ap['lp'][:, n * N:(n + 1) * N], in_=lpo[:])

    nc.compile()
    return nc


def _get_nc(consts, meta):
    if 'nc' not in _CACHE:
        const_shapes = {k: v.shape for k, v in consts.items()}
        _CACHE['nc'] = _build_nc(const_shapes, meta)
    return _CACHE['nc']


def kernel(**inputs):
    from concourse.bass_utils import run_bass_kernel_spmd

    consts, meta = _build_consts(inputs)
    nc = _get_nc(consts, meta)

    x = _f32(inputs['input'])
    eps = _f32(inputs['eps'])
    emb = _f32(inputs['emb'])
    cond = np.asarray(inputs['condition'])
    c_emb = emb[cond]                       # [B, CD] host gather (10-row table)

    in_maps = []
    for i in range(NCORES):
        sl = slice(i * BL, (i + 1) * BL)
        xp = np.zeros((D_PAD, BL), dtype=np.float32)
        xp[:D_IN] = x[sl].T
        xT = np.ascontiguousarray(
            xp.reshape(KC0, 128, BL).transpose(1, 0, 2).reshape(128, KC0 * BL))
        m = {
            'xT': xT,
            'epsT': np.ascontiguousarray(eps[sl].T),
            'cT': np.ascontiguousarray(c_emb[sl].T),
        }
        m.update(consts)
        in_maps.append(m)

    res = run_bass_kernel_spmd(nc, in_maps, core_ids=list(range(NCORES)),
                               trace=TRACE)
    global LAST_EXEC_NS, LAST_RES
    LAST_EXEC_NS = res.exec_time_ns
    LAST_RES = res

    rec = np.concatenate([res.results[i]['recT'].T for i in range(NCORES)], axis=0)
    mean = np.concatenate([res.results[i]['meanT'].T for i in range(NCORES)], axis=0)
    std = np.concatenate([res.results[i]['stdT'].T for i in range(NCORES)], axis=0)
    lp = np.concatenate([res.results[i]['lp'][0] for i in range(NCORES)], axis=0)

    side = int(D_IN ** 0.5)
    reconstructed = np.ascontiguousarray(rec.reshape(B, 1, side, side))
    x_probs = np.float32(np.mean(lp))
    return reconstructed, x_probs, mean, std


# revision 14
# speedup vs baseline: 1.9707x; 1.9707x over previous
"""AECNF (autoencoder + conditional CNF) forward pass on 8 TRN2 NeuronCores.

Strategy: pure data parallelism — batch 8192 is split into 8 shards of 1024,
one per core; all parameters are replicated. On-device everything is kept
feature-major ([features, batch]) so the whole network is a chain of
matmuls with no transposes: out = W @ X via nc.tensor.matmul(psum, lhsT, rhs)
with lhsT = W.T (features on partitions).

The HyperNetwork depends only on t (33 unique compile-time-constant values),
not on the batch, so its outputs (Wm, Um*sigmoid(gate), Bm, wu) are
precomputed on the host as weight preprocessing and fed to the device as
packed constant tensors. The condition path dc/dt = c is linear, so
c(t) = c_emb * factor(t) with host-precomputed f32 scalar factors folded
into the per-eval matmul weights.

The log-det trace term accumulates across all 64 ODE evals directly in a
persistent PSUM bank: logp_x = PSUM(sum_e -cw_e/256 * wu_e . hh_e^2
                                     - 0.5 * 1 . z0^2) + CONST.
"""
import sys
import numpy as np

if '/opt/trn_rl_repo' not in sys.path:
    sys.path.insert(0, '/opt/trn_rl_repo')

# ---- problem dims (hardcoded per spec) ----
B = 8192
NCORES = 8
BL = B // NCORES          # 1024 per core
N = 512                   # batch chunk (PSUM free-dim limit)
NCH = BL // N             # 2 chunks
D_IN = 784
H = 512
LD = 16
CD = 32
WID = 256
OH = 256
NSTEPS = 16
KC0 = 7                   # ceil(784/128)
D_PAD = KC0 * 128         # 896
MC0 = H // 128            # 4
KC1 = H // 128            # 4
MC2 = KC0                 # 7 output chunks for decoder L3
NU = 2 * NSTEPS + 1       # 33 unique hypernet t values
NE = 4 * NSTEPS           # 64 ODE function evals

_F32 = np.float32

_CACHE = {}
TRACE = False          # set by test harness to capture a neuron-profile trace
LAST_EXEC_NS = None
LAST_RES = None


def _f32(x):
    return np.asarray(x, dtype=np.float32)


def _sigmoid32(x):
    return (1.0 / (1.0 + np.exp(-x.astype(np.float32)))).astype(np.float32)


def _hypernet(t, hw1, hb1, hw2, hb2, hw3, hb3):
    """Host replica of the reference HyperNetwork at scalar t (float32)."""
    x = np.tanh(t * hw1[:, 0] + hb1).astype(np.float32)
    x = np.tanh(x @ hw2.T + hb2).astype(np.float32)
    p = (x @ hw3.T + hb3).astype(np.float32)
    bs = WID * LD
    Wm = p[:bs].reshape(WID, LD)
    Um = (p[bs:2 * bs].reshape(WID, LD) * _sigmoid32(p[2 * bs:3 * bs].reshape(WID, LD))).astype(np.float32)
    Bm = p[3 * bs:]
    wu = np.sum(Wm * Um, axis=1, dtype=np.float32)
    return Wm, Um, Bm, wu


def _pad_ms_cols(wT):
    """[K, 32] -> [K, 48]: cols 0:16 kept, 16:32 zero, 32:48 = old 16:32."""
    out = np.zeros((wT.shape[0], 48), dtype=np.float32)
    out[:, 0:LD] = wT[:, 0:LD]
    out[:, 32:48] = wT[:, LD:2 * LD]
    return out


def _pad_ms_rows(b):
    out = np.zeros((48, 1), dtype=np.float32)
    out[0:LD, 0] = b[:LD]
    out[32:48, 0] = b[LD:2 * LD]
    return out


def _pack_k(wT, kc, m):
    """Pack a [K, M] lhsT (K possibly >128, zero-padded to kc*128) into
    [128, kc*m] so slice [:, c*m + j] gives K-chunk c."""
    K, M = wT.shape
    assert M == m
    buf = np.zeros((kc * 128, M), dtype=np.float32)
    buf[:K] = wT
    return np.ascontiguousarray(buf.reshape(kc, 128, M).transpose(1, 0, 2).reshape(128, kc * M))


def _build_consts(ins):
    """All host-side weight preprocessing. Returns dict of device arrays."""
    g = {k: _f32(v) for k, v in ins.items() if k != 'condition'}

    dt = (_F32(0.0) - _F32(10.0)) / _F32(NSTEPS)       # -0.625
    hdt = dt / _F32(2.0)
    sdt6 = dt / _F32(6.0)

    # unique hypernet ts: t_u = 10 + u * dt/2
    t_us = [_F32(10.0) + hdt * _F32(u) for u in range(NU)]
    hyper = [_hypernet(t, g['hw1'], g['hb1'], g['hw2'], g['hb2'], g['hw3'], g['hb3'])
             for t in t_us]

    def u_of(step, stage):
        return 2 * step + (0, 1, 1, 2)[stage]

    # c(t) factors: f32 sequential RK4 on dc/dt = c
    cfac = np.zeros(NE, dtype=np.float32)
    fn = _F32(1.0)
    for step in range(NSTEPS):
        f1 = fn
        f2 = fn + hdt * f1
        f3 = fn + hdt * f2
        f4 = fn + dt * f3
        cfac[step * 4 + 0] = f1
        cfac[step * 4 + 1] = f2
        cfac[step * 4 + 2] = f3
        cfac[step * 4 + 3] = f4
        fn = fn + sdt6 * (f1 + _F32(2.0) * f2 + _F32(2.0) * f3 + f4)

    # per-eval a-matmul weights: lhsT = [cwz | cwc*cfac].T, output cols padded
    # so the sigmoid half lands at PSUM partition 32 (PSUM reads must be
    # 32-partition aligned): cols 0:16 tanh-half, 16:32 zero, 32:48 sig-half.
    aW = np.zeros((CD + LD, NE * 48), dtype=np.float32)
    for e in range(NE):
        blk = np.zeros((CD + LD, 48), dtype=np.float32)
        blk[:LD, 0:LD] = g['cwz'].T[:, 0:LD]
        blk[LD:, 0:LD] = (g['cwc'] * cfac[e]).T[:, 0:LD]
        blk[:LD, 32:48] = g['cwz'].T[:, LD:]
        blk[LD:, 32:48] = (g['cwc'] * cfac[e]).T[:, LD:]
        aW[:, e * 48:(e + 1) * 48] = blk
    b0c = (g['cbz'] + g['cbc']).astype(np.float32)
    b0 = np.zeros((48, 1), dtype=np.float32)
    b0[0:LD, 0] = b0c[:LD]
    b0[32:48, 0] = b0c[LD:]

    # hypernet-derived packs
    hhW = np.zeros((LD, NU * WID), dtype=np.float32)
    hhB = np.zeros((128, NU * 2), dtype=np.float32)
    kW = np.zeros((128, NU * 2 * LD), dtype=np.float32)
    for u, (Wm, Um, Bm, _) in enumerate(hyper):
        hhW[:, u * WID:(u + 1) * WID] = Wm.T
        for h in range(2):
            hhB[:, u * 2 + h] = Bm[h * 128:(h + 1) * 128]
            kW[:, u * 32 + h * LD:(u + 1 - 1) * 32 + (h + 1) * LD] = \
                Um[h * 128:(h + 1) * 128] / _F32(WID)

    # logp accumulation weights + constant
    w_stage = [sdt6 * _F32(m) for m in (1.0, 2.0, 2.0, 1.0)]
    lpW = np.zeros((128, NE * 2), dtype=np.float32)
    CONST = _F32(0.0)
    for step in range(NSTEPS):
        for stage in range(4):
            e = step * 4 + stage
            u = u_of(step, stage)
            wu = hyper[u][3]
            sc = -(w_stage[stage] / _F32(WID))
            for h in range(2):
                lpW[:, e * 2 + h] = wu[h * 128:(h + 1) * 128] * sc
            S0 = np.sum(wu, dtype=np.float32)
            CONST = CONST + w_stage[stage] * S0 / _F32(WID)
    CONST = _F32(CONST - _F32(0.5 * LD) * _F32(np.log(2 * np.pi)))
    z0W = np.full((LD, 1), -0.5, dtype=np.float32)

    consts = {
        'ew0p': _pack_k(g['ew0'].T, KC0, H),
        'ew1p': _pack_k(g['ew1'].T, KC1, H),
        'ew2p': _pack_k(_pad_ms_cols(g['ew2'].T), KC1, 48),
        'eb0p': np.ascontiguousarray(g['eb0'].reshape(MC0, 128).T),
        'eb1p': np.ascontiguousarray(g['eb1'].reshape(MC0, 128).T),
        'eb2p': _pad_ms_rows(g['eb2']),
        'dw0p': np.ascontiguousarray(g['dw0'].T),           # [16, 512]
        'dw1p': _pack_k(g['dw1'].T, KC1, H),
        'dw2p': _pack_k(g['dw2'].T, KC1, D_IN),
        'db0p': np.ascontiguousarray(g['db0'].reshape(MC0, 128).T),
        'db1p': np.ascontiguousarray(g['db1'].reshape(MC0, 128).T),
        'db2p': np.ascontiguousarray(
            np.pad(g['db2'], (0, D_PAD - D_IN)).reshape(MC2, 128).T),
        'aW': aW, 'b0': b0, 'hhW': hhW, 'hhB': hhB, 'kW': kW,
        'lpW': lpW, 'z0W': z0W,
    }
    meta = {
        'CONST': float(CONST),
        'w_stage': [float(w) for w in w_stage],
        's_fac': [float(hdt), float(hdt), float(dt)],
        'u_of': u_of,
    }
    return consts, meta


def _build_nc(const_shapes, meta):
    import concourse.bass as bass
    import concourse.tile as tile
    from concourse import bacc, mybir

    AF = mybir.ActivationFunctionType
    ALU = mybir.AluOpType
    f32 = mybir.dt.float32

    nc = bacc.Bacc("TRN2", target_bir_lowering=False, debug=False,
                   num_devices=NCORES)

    f32r_early = mybir.dt.float32r
    # consts that feed matmuls are declared float32r end-to-end
    MMW = {'ew0p', 'ew1p', 'ew2p', 'dw0p', 'dw1p', 'dw2p', 'aW', 'hhW', 'kW',
           'lpW', 'z0W'}
    ap = {}
    ap['xT'] = nc.dram_tensor("xT", [128, KC0 * BL], f32r_early, kind="ExternalInput").ap()
    ap['epsT'] = nc.dram_tensor("epsT", [LD, BL], f32, kind="ExternalInput").ap()
    ap['cT'] = nc.dram_tensor("cT", [CD, BL], f32r_early, kind="ExternalInput").ap()
    for k, shp in const_shapes.items():
        dt_k = f32r_early if k in MMW else f32
        ap[k] = nc.dram_tensor(k, list(shp), dt_k, kind="ExternalInput").ap()
    ap['recT'] = nc.dram_tensor("recT", [D_IN, BL], f32, kind="ExternalOutput").ap()
    ap['meanT'] = nc.dram_tensor("meanT", [LD, BL], f32, kind="ExternalOutput").ap()
    ap['stdT'] = nc.dram_tensor("stdT", [LD, BL], f32, kind="ExternalOutput").ap()
    ap['lp'] = nc.dram_tensor("lp", [1, BL], f32, kind="ExternalOutput").ap()

    f32r = mybir.dt.float32r

    def MM(out, lhsT, rhs, **kw):
        # float32r: bitwise-f32 storage, single-pass PE (~3x faster than f32's
        # two half-speed passes), ~1.5e-4 multiply precision.
        return nc.tensor.matmul(out, lhsT.bitcast(f32r), rhs.bitcast(f32r), **kw)

    ACT = nc.scalar.activation
    CONST = meta['CONST']
    w_stage = meta['w_stage']
    s_fac = meta['s_fac']
    u_of = meta['u_of']

    with tile.TileContext(nc) as tc:
        from contextlib import ExitStack
        with ExitStack() as ctx:
            wp = ctx.enter_context(tc.tile_pool(name="wts", bufs=1))
            sp = ctx.enter_context(tc.tile_pool(name="state", bufs=1))
            lpp = ctx.enter_context(tc.tile_pool(name="lp_ps", bufs=1, space="PSUM"))

            # ---- load all weights/constants once ----
            W = {}
            for k, shp in const_shapes.items():
                dt_k = f32r if k in MMW else f32
                W[k] = wp.tile(list(shp), dt_k, tag=k, name=f"w_{k}")
                nc.sync.dma_start(out=W[k][:], in_=ap[k][:])

            # ---- persistent state ----
            X = sp.tile([CD + LD, BL], f32r, tag="X", name="X")
            X2 = sp.tile([CD + LD, BL], f32r, tag="X2", name="X2")
            zacc = [sp.tile([LD, N], f32, tag=f"zacc{n}", name=f"zacc{n}") for n in range(NCH)]
            z_sb = sp.tile([LD, BL], f32r, tag="z_sb", name="z_sb")
            mean_sb = sp.tile([LD, BL], f32, tag="mean_sb", name="mean_sb")
            std_sb = sp.tile([LD, BL], f32, tag="std_sb", name="std_sb")
            eps_sb = sp.tile([LD, BL], f32, tag="eps_sb", name="eps_sb")
            lp_ps = [lpp.tile([1, N], f32, tag=f"lp{n}", name=f"lp{n}") for n in range(NCH)]

            nc.sync.dma_start(out=eps_sb[:], in_=ap['epsT'][:])
            nc.sync.dma_start(out=X[LD:, :], in_=ap['cT'][:])
            nc.sync.dma_start(out=X2[LD:, :], in_=ap['cT'][:])

            # ================= encoder =================
            with tc.tile_pool(name="encps", bufs=1, space="PSUM") as psE, \
                 tc.tile_pool(name="enc_sb", bufs=2) as esb, \
                 tc.tile_pool(name="acts", bufs=1) as actp:
                h0 = actp.tile([128, MC0 * BL], f32r, tag="h0", name="h0")
                h1 = actp.tile([128, MC0 * BL], f32r, tag="h1", name="h1")

                def elu_store(ps, bias_ap, dst):
                    ee = esb.tile([128, N], f32, tag="ee", name="ee", bufs=2)
                    ACT(ee[:], ps[:], AF.Exp, bias=bias_ap)
                    rr = esb.tile([128, N], f32, tag="rr", name="rr", bufs=2)
                    ACT(rr[:], ps[:], AF.Relu, bias=bias_ap)
                    t2 = esb.tile([128, N], f32, tag="t2", name="t2", bufs=2)
                    ACT(t2[:], ee[:], AF.Relu, scale=-1.0, bias=1.0)
                    nc.vector.tensor_sub(dst, rr[:], t2[:])

                # L1: h0 = elu(ew0 @ x + eb0)
                for n in range(NCH):
                    ps_l = [psE.tile([128, N], f32, tag=f"eps{m}", name=f"eps{m}_{n}")
                            for m in range(MC0)]
                    for kc in range(KC0):
                        xk = esb.tile([128, N], f32r, tag="xk", name="xk", bufs=3)
                        nc.sync.dma_start(
                            out=xk[:], in_=ap['xT'][:, kc * BL + n * N: kc * BL + (n + 1) * N])
                        for m in range(MC0):
                            MM(ps_l[m][:], W['ew0p'][:, kc * H + m * 128: kc * H + (m + 1) * 128],
                               xk[:], start=(kc == 0), stop=(kc == KC0 - 1))
                    for m in range(MC0):
                        elu_store(ps_l[m], W['eb0p'][:, m:m + 1],
                                  h0[:, m * BL + n * N: m * BL + (n + 1) * N])

                # L2: h1 = tanh(ew1 @ h0 + eb1)
                for n in range(NCH):
                    ps_l = [psE.tile([128, N], f32, tag=f"eps{m}", name=f"eps2{m}_{n}")
                            for m in range(MC0)]
                    for kc in range(KC1):
                        for m in range(MC0):
                            MM(ps_l[m][:], W['ew1p'][:, kc * H + m * 128: kc * H + (m + 1) * 128],
                               h0[:, kc * BL + n * N: kc * BL + (n + 1) * N],
                               start=(kc == 0), stop=(kc == KC1 - 1))
                    for m in range(MC0):
                        ACT(h1[:, m * BL + n * N: m * BL + (n + 1) * N],
                            ps_l[m][:], AF.Tanh, bias=W['eb1p'][:, m:m + 1])

                # L3: ms = ew2 @ h1 + eb2; mean/std; z = mean + eps*std
                for n in range(NCH):
                    ps3 = psE.tile([48, N], f32, tag="eps_ms", name=f"eps_ms{n}")
                    for kc in range(KC1):
                        MM(ps3[:], W['ew2p'][:, kc * 48:(kc + 1) * 48],
                           h1[:, kc * BL + n * N: kc * BL + (n + 1) * N],
                           start=(kc == 0), stop=(kc == KC1 - 1))
                    ACT(mean_sb[:, n * N:(n + 1) * N], ps3[0:LD, :], AF.Identity,
                        bias=W['eb2p'][0:LD, 0:1])
                    ACT(std_sb[:, n * N:(n + 1) * N], ps3[32:48, :], AF.Exp,
                        bias=W['eb2p'][32:48, 0:1])
                nc.vector.tensor_scalar_add(std_sb[:], std_sb[:], 1e-6)
                nc.vector.tensor_mul(z_sb[:], eps_sb[:], std_sb[:])
                nc.vector.tensor_add(z_sb[:], z_sb[:], mean_sb[:])
                nc.vector.tensor_copy(X[0:LD, :], z_sb[:])
                nc.sync.dma_start(out=ap['meanT'][:], in_=mean_sb[:])
                nc.sync.dma_start(out=ap['stdT'][:], in_=std_sb[:])

            # ================= decoder =================
            with tc.tile_pool(name="decps", bufs=1, space="PSUM") as psD, \
                 tc.tile_pool(name="dec_sb", bufs=2) as dsb, \
                 tc.tile_pool(name="dacts", bufs=1) as dactp:
                d1 = dactp.tile([128, MC0 * BL], f32r, tag="d1", name="d1")
                d2 = dactp.tile([128, MC0 * BL], f32r, tag="d2", name="d2")

                def delu_store(ps, bias_ap, dst):
                    ee = dsb.tile([128, N], f32, tag="dee", name="dee", bufs=2)
                    ACT(ee[:], ps[:], AF.Exp, bias=bias_ap)
                    rr = dsb.tile([128, N], f32, tag="drr", name="drr", bufs=2)
                    ACT(rr[:], ps[:], AF.Relu, bias=bias_ap)
                    t2 = dsb.tile([128, N], f32, tag="dt2", name="dt2", bufs=2)
                    ACT(t2[:], ee[:], AF.Relu, scale=-1.0, bias=1.0)
                    nc.vector.tensor_sub(dst, rr[:], t2[:])

                # L1: d1 = elu(dw0 @ z + db0)   (K = 16, single chunk)
                for n in range(NCH):
                    for m in range(MC0):
                        ps = psD.tile([128, N], f32, tag=f"dps{m}", name=f"dps{m}_{n}")
                        MM(ps[:], W['dw0p'][:, m * 128:(m + 1) * 128],
                           z_sb[:, n * N:(n + 1) * N], start=True, stop=True)
                        delu_store(ps, W['db0p'][:, m:m + 1],
                                   d1[:, m * BL + n * N: m * BL + (n + 1) * N])

                # L2: d2 = tanh(dw1 @ d1 + db1)
                for n in range(NCH):
                    ps_l = [psD.tile([128, N], f32, tag=f"dps{m}", name=f"dps2{m}_{n}")
                            for m in range(MC0)]
                    for kc in range(KC1):
                        for m in range(MC0):
                            MM(ps_l[m][:], W['dw1p'][:, kc * H + m * 128: kc * H + (m + 1) * 128],
                               d1[:, kc * BL + n * N: kc * BL + (n + 1) * N],
                               start=(kc == 0), stop=(kc == KC1 - 1))
                    for m in range(MC0):
                        ACT(d2[:, m * BL + n * N: m * BL + (n + 1) * N],
                            ps_l[m][:], AF.Tanh, bias=W['db1p'][:, m:m + 1])

                # L3: rec = sigmoid(dw2 @ d2 + db2)
                for n in range(NCH):
                    for m in range(MC2):
                        mm = 128 if m < MC2 - 1 else D_IN - 128 * (MC2 - 1)
                        ps = psD.tile([128, N], f32, tag=f"dps{m % MC0}",
                                      name=f"dps3{m}_{n}")
                        for kc in range(KC1):
                            MM(ps[0:mm, :],
                               W['dw2p'][:, kc * D_IN + m * 128: kc * D_IN + m * 128 + mm],
                               d2[:, kc * BL + n * N: kc * BL + (n + 1) * N],
                               start=(kc == 0), stop=(kc == KC1 - 1))
                        rec = dsb.tile([128, N], f32, tag="rec", name=f"rec{m}_{n}", bufs=3)
                        ACT(rec[0:mm, :], ps[0:mm, :], AF.Sigmoid,
                            bias=W['db2p'][0:mm, m:m + 1])
                        nc.sync.dma_start(
                            out=ap['recT'][m * 128:m * 128 + mm, n * N:(n + 1) * N],
                            in_=rec[0:mm, :])

            # ================= ODE (RK4, 16 steps, 64 evals) =================
            with tc.tile_pool(name="psA", bufs=2, space="PSUM") as psA, \
                 tc.tile_pool(name="psH", bufs=2, space="PSUM") as psH, \
                 tc.tile_pool(name="psK", bufs=2, space="PSUM") as psK, \
                 tc.tile_pool(name="ode_sb", bufs=3) as tp, \
                 tc.tile_pool(name="hh_sb", bufs=4) as hp:

                for step in range(NSTEPS):
                    for stage in range(4):
                        e = step * 4 + stage
                        u = u_of(step, stage)
                        for n in range(NCH):
                            rhs_t = X if stage == 0 else X2
                            rhs = rhs_t[:, n * N:(n + 1) * N]
                            aps = psA.tile([48, N], f32, tag="aps", name=f"aps{e}_{n}")
                            MM(aps[:], W['aW'][:, e * 48:(e + 1) * 48], rhs,
                               start=True, stop=True)
                            t1 = tp.tile([LD, N], f32, tag="t1", name=f"t1_{e}_{n}")
                            ACT(t1[:], aps[0:LD, :], AF.Tanh, bias=W['b0'][0:LD, 0:1])
                            s1 = tp.tile([LD, N], f32, tag="s1", name=f"s1_{e}_{n}")
                            ACT(s1[:], aps[32:48, :], AF.Sigmoid,
                                bias=W['b0'][32:48, 0:1])
                            zc = tp.tile([LD, N], f32r, tag="zc", name=f"zc_{e}_{n}")
                            nc.vector.tensor_mul(zc[:], t1[:], s1[:])

                            kps = psK.tile([LD, N], f32, tag="kps", name=f"kps{e}_{n}")
                            for h in range(2):
                                hps = psH.tile([128, N], f32, tag="hps",
                                               name=f"hps{e}_{n}_{h}")
                                MM(hps[:], W['hhW'][:, u * WID + h * 128: u * WID + (h + 1) * 128],
                                   zc[:], start=True, stop=True)
                                hh = hp.tile([128, N], f32r, tag="hh",
                                             name=f"hh{e}_{n}_{h}", bufs=4)
                                ACT(hh[:], hps[:], AF.Tanh,
                                    bias=W['hhB'][:, u * 2 + h: u * 2 + h + 1])
                                ss = hp.tile([128, N], f32r, tag="ss",
                                             name=f"ss{e}_{n}_{h}", bufs=4)
                                nc.vector.tensor_mul(ss[:], hh[:], hh[:])
                                MM(kps[:], W['kW'][:, u * 32 + h * LD: u * 32 + (h + 1) * LD],
                                   hh[:], start=(h == 0), stop=(h == 1))
                                MM(lp_ps[n][:], W['lpW'][:, e * 2 + h: e * 2 + h + 1],
                                   ss[:], start=(e == 0 and h == 0), stop=False,
                                   skip_group_check=True)

                            Xz = X[0:LD, n * N:(n + 1) * N]
                            if stage == 0:
                                nc.vector.scalar_tensor_tensor(
                                    zacc[n][:], kps[:], w_stage[0], Xz,
                                    op0=ALU.mult, op1=ALU.add)
                            elif stage < 3:
                                nc.vector.scalar_tensor_tensor(
                                    zacc[n][:], kps[:], w_stage[stage], zacc[n][:],
                                    op0=ALU.mult, op1=ALU.add)
                            else:
                                nc.vector.scalar_tensor_tensor(
                                    Xz, kps[:], w_stage[3], zacc[n][:],
                                    op0=ALU.mult, op1=ALU.add)
                            if stage < 3:
                                nc.vector.scalar_tensor_tensor(
                                    X2[0:LD, n * N:(n + 1) * N], kps[:], s_fac[stage], Xz,
                                    op0=ALU.mult, op1=ALU.add)

                # ---- finish logp: -0.5*sum(z0^2), add CONST, write out ----
                for n in range(NCH):
                    sq = tp.tile([LD, N], f32r, tag="sq", name=f"sq{n}", bufs=2)
                    nc.vector.tensor_mul(sq[:], X[0:LD, n * N:(n + 1) * N],
                                         X[0:LD, n * N:(n + 1) * N])
                    MM(lp_ps[n][:], W['z0W'][:], sq[:], start=False, stop=True,
                       skip_group_check=True)
                    lpo = tp.tile([1, N], f32, tag="lpo", name=f"lpo{n}", bufs=2)
                    ACT(lpo[:], lp_ps[n][:], AF.Copy, bias=CONST)
                    nc.sync.dma_start(out=ap['lp'][:, n * N:(n + 1) * N], in_=lpo[:])

    nc.compile()
    return nc


def _get_nc(consts, meta):
    if 'nc' not in _CACHE:
        const_shapes = {k: v.shape for k, v in consts.items()}
        _CACHE['nc'] = _build_nc(const_shapes, meta)
    return _CACHE['nc']


def kernel(**inputs):
    from concourse.bass_utils import run_bass_kernel_spmd

    consts, meta = _build_consts(inputs)
    nc = _get_nc(consts, meta)

    x = _f32(inputs['input'])
    eps = _f32(inputs['eps'])
    emb = _f32(inputs['emb'])
    cond = np.asarray(inputs['condition'])
    c_emb = emb[cond]                       # [B, CD] host gather (10-row table)

    in_maps = []
    for i in range(NCORES):
        sl = slice(i * BL, (i + 1) * BL)
        xp = np.zeros((D_PAD, BL), dtype=np.float32)
        xp[:D_IN] = x[sl].T
        xT = np.ascontiguousarray(
            xp.reshape(KC0, 128, BL).transpose(1, 0, 2).reshape(128, KC0 * BL))
        m = {
            'xT': xT,
            'epsT': np.ascontiguousarray(eps[sl].T),
            'cT': np.ascontiguousarray(c_emb[sl].T),
        }
        m.update(consts)
        in_maps.append(m)

    res = run_bass_kernel_spmd(nc, in_maps, core_ids=list(range(NCORES)),
                               trace=TRACE)
    global LAST_EXEC_NS, LAST_RES
    LAST_EXEC_NS = res.exec_time_ns
    LAST_RES = res

    rec = np.concatenate([res.results[i]['recT'].T for i in range(NCORES)], axis=0)
    mean = np.concatenate([res.results[i]['meanT'].T for i in range(NCORES)], axis=0)
    std = np.concatenate([res.results[i]['stdT'].T for i in range(NCORES)], axis=0)
    lp = np.concatenate([res.results[i]['lp'][0] for i in range(NCORES)], axis=0)

    side = int(D_IN ** 0.5)
    reconstructed = np.ascontiguousarray(rec.reshape(B, 1, side, side))
    x_probs = np.float32(np.mean(lp))
    return reconstructed, x_probs, mean, std
